# revision 1
# baseline (speedup 1.0000x reference)
"""Trainium2 Bass kernel for a 3-layer GCN (ExtendedGCN).

Math (per reference):
    agg(F) = D^-1/2 (A + I) D^-1/2 F      with deg = in-degree + 1
    Z1 = agg(x) @ W1 + b1 ; H1 = relu(Z1)
    Z2 = agg(H1) @ W2 + b2
    Z3 = agg(H2=Z2) @ W3 + b3 ; out = softmax(Z3, axis=1)
(aggregate-then-project is exact: message passing commutes with the
right-multiplication by W).

Distribution: nodes are partitioned across 8 cores (dst-owner edge split).
Each layer, every core computes its own node rows, then the scaled feature
table X̂ = dinv ⊙ H is AllGathered so every core can gather arbitrary source
rows locally.  Per-node contributor lists (in-neighbors + self-loop) are
precomputed on the host as table-row indices, grouped per 128-node block so a
single indirect DMA gathers a [128, K, D] tile and a short in-place tree of
vector adds produces the aggregate.

Folding of the symmetric normalization: with X̂_l = dinv ⊙ H_l as the gather
table, S = plain sum of gathered rows (self-loop included as an ordinary
slot), the next table is directly
    X̂_{l+1} = relu?( (dinv^2 ⊙ S) @ W_l + dinv*b_l )
and the final logits are Z3 = (dinv ⊙ S3) @ W3 + b3.
"""

import sys

sys.path.insert(0, "/opt/trn_rl_repo")

import numpy as np

N_CORES = 8
P = 128  # partitions / block size
BF16_TABLES = False  # bf16 feature tables: halves gather + all-gather traffic
GATHER_MODE = "indirect"  # "indirect" (per-column indirect DMA) | "bulk" (dma_gather)
ABLATE = ""  # dev-only: "ag" skips collectives, "gather" skips table gathers


# --------------------------------------------------------------------------
# Host-side graph preprocessing (integer index work only)
# --------------------------------------------------------------------------
def preprocess(edge_index, n_nodes, n_cores=N_CORES):
    src = np.asarray(edge_index[0]).astype(np.int64)
    dst = np.asarray(edge_index[1]).astype(np.int64)

    deg = np.bincount(dst, minlength=n_nodes).astype(np.int64) + 1  # + self

    # order nodes by degree (desc) so blocks have uniform slot counts
    order = np.argsort(-deg, kind="stable")  # sorted position k -> node id
    chunk = P * n_cores
    n_pad = ((n_nodes + chunk - 1) // chunk) * chunk
    J = n_pad // chunk  # blocks per core
    ZROW = n_pad  # index of the all-zero table row

    k = np.arange(n_pad)
    g = k // P  # global block
    core_of_k = g % n_cores
    jj_of_k = g // n_cores
    row_of_k = core_of_k * (J * P) + jj_of_k * P + (k % P)

    rank = np.empty(n_nodes, dtype=np.int64)
    rank[order] = np.arange(n_nodes)
    row_of_node = row_of_k[rank]  # node id -> table row

    deg_sorted = deg[order]  # desc
    K_u = []
    for jj in range(J):
        k0 = jj * chunk
        K_u.append(int(deg_sorted[k0]) if k0 < n_nodes else 1)
    S = int(np.sum(K_u))
    off = np.concatenate([[0], np.cumsum(K_u)[:-1]]).astype(np.int64)

    # slot lists: idx[core, p, off[jj]+s] = table row of s-th contributor
    idx = np.full((n_cores, P, S), ZROW, dtype=np.int32)

    # self-loop entries (slot 0) for real nodes
    kr = rank  # k of each real node
    idx[core_of_k[kr], kr % P, off[jj_of_k[kr]]] = row_of_node.astype(np.int32)

    # edge entries, slots 1..cnt
    er = rank[dst]  # sorted-position of each edge's dst
    eorder = np.argsort(er, kind="stable")
    er_s = er[eorder]
    src_rows = row_of_node[src[eorder]].astype(np.int32)
    cnt = np.bincount(er_s, minlength=n_pad)
    start = np.concatenate([[0], np.cumsum(cnt)[:-1]])
    slot = np.arange(len(er_s)) - start[er_s] + 1
    col = off[jj_of_k[er_s]] + slot
    idx[core_of_k[er_s], er_s % P, col] = src_rows

    # per-core degree array [P, J] (deg of local node (jj,p) at [p,jj])
    deg_by_row = np.ones(n_pad, dtype=np.float32)
    deg_by_row[row_of_node] = deg.astype(np.float32)
    deg_arr = deg_by_row.reshape(n_cores, J, P).transpose(0, 2, 1).copy()
    # row layout [1, J*P] (deg of local node (jj,p) at [0, jj*P+p])
    deg_row = deg_by_row.reshape(n_cores, 1, J * P).copy()

    return dict(
        n_pad=n_pad,
        J=J,
        S=S,
        K_u=K_u,
        off=off,
        idx=idx,
        deg_arr=deg_arr,
        deg_row=deg_row,
        row_of_node=row_of_node,
        core_of_node=row_of_node // (J * P),
        local_of_node=row_of_node % (J * P),
        idx_key="gidx",
    )


def preprocess2(edge_index, n_nodes, n_cores=N_CORES):
    """Host preprocessing for the bulk dma_gather path.

    Table layout: 8 per-core slices of SLICE = J*128+1 rows each; the last
    row of every slice is all-zeros (gather target for padding).  int16
    index limit: the lo half = first 5 slices (rows [0, 5*SLICE)), hi half =
    remaining 3 slices; per (block, half) the per-node slot lists are padded
    to the block's max count, indices stored half-relative in the wrapped
    [16]-partition int16 layout dma_gather expects.
    """
    src = np.asarray(edge_index[0]).astype(np.int64)
    dst = np.asarray(edge_index[1]).astype(np.int64)
    n = n_nodes
    deg = np.bincount(dst, minlength=n).astype(np.int64) + 1

    chunk = P * n_cores
    n_pad = ((n + chunk - 1) // chunk) * chunk
    J = n_pad // chunk
    SLICE = J * P + 1
    R = n_cores * SLICE
    N_LO = 5
    B = N_LO * SLICE  # lo/hi boundary row
    assert B - 1 <= 32767 and R - B - 1 <= 32767

    S_all = np.concatenate([src, np.arange(n)])
    D_all = np.concatenate([dst, np.arange(n)])

    def lo_of_rank(r):
        return (r // P) % n_cores < N_LO

    order = np.argsort(-deg, kind="stable")
    for _ in range(2):
        rank = np.empty(n, np.int64)
        rank[order] = np.arange(n)
        is_lo = lo_of_rank(rank[S_all])
        lo = np.zeros(n, np.int64)
        np.add.at(lo, D_all, is_lo)
        hi = deg - lo
        order = np.lexsort((-hi, -lo))
    rank = np.empty(n, np.int64)
    rank[order] = np.arange(n)
    is_lo = lo_of_rank(rank[S_all])
    lo = np.zeros(n, np.int64)
    np.add.at(lo, D_all, is_lo)
    hi = deg - lo

    # rank -> (core, jj, p) -> table row
    def row_of_rank(r):
        g = r // P
        return (g % n_cores) * SLICE + (g // n_cores) * P + (r % P)

    row_of_node = row_of_rank(rank)

    # uniform per-block-index slot counts (max over the 8 cores)
    lo_pad = np.zeros(n_pad, np.int64)
    lo_pad[rank] = lo
    hi_pad = np.zeros(n_pad, np.int64)
    hi_pad[rank] = hi
    K_A = [int(lo_pad[jj * chunk : (jj + 1) * chunk].max()) for jj in range(J)]
    K_B = [int(hi_pad[jj * chunk : (jj + 1) * chunk].max()) for jj in range(J)]
    S2 = int(np.sum(K_A) + np.sum(K_B))

    # gi16 [cores, 128, 8*S2] prefilled with the zero-row relative index
    ZREL = J * P  # 6272 both halves (core0-zero for lo, core(N_LO)-zero for hi)
    gi16 = np.full((n_cores, 16, 8 * S2), ZREL, dtype=np.int16)
    col0 = np.zeros((J, 2), np.int64)  # column offset (in slot cols) per (jj, half)
    acc = 0
    for jj in range(J):
        col0[jj, 0] = acc
        acc += K_A[jj]
        col0[jj, 1] = acc
        acc += K_B[jj]

    er = rank[D_all]  # dst rank of each (edge incl self)
    src_row = row_of_node[S_all]
    for half in (0, 1):
        sel = np.where(is_lo if half == 0 else ~is_lo)[0]
        ers = er[sel]
        eorder = np.argsort(ers, kind="stable")
        ers = ers[eorder]
        rows = src_row[sel][eorder] - (0 if half == 0 else B)
        cnt = np.bincount(ers, minlength=n_pad)
        start = np.concatenate([[0], np.cumsum(cnt)[:-1]])
        s = np.arange(len(ers)) - start[ers]
        g = ers // P
        c = g % n_cores
        jjv = g // n_cores
        p = ers % P
        j = s * P + p  # index position within the call
        col = col0[jjv, half] * 8 + j // 16
        gi16[c, j % 16, col] = rows.astype(np.int16)
    gi16 = np.tile(gi16, (1, 8, 1))  # replicate 16-row wrap to 128 partitions

    deg_by_rank = np.ones(n_pad, dtype=np.float32)
    deg_by_rank[rank] = deg.astype(np.float32)
    deg_arr = deg_by_rank.reshape(J, n_cores, P).transpose(1, 2, 0).copy()
    deg_row = deg_by_rank.reshape(J, n_cores, P).transpose(1, 0, 2).reshape(
        n_cores, 1, J * P
    ).copy()

    return dict(
        n_pad=n_pad, J=J, SLICE=SLICE, R=R, B=B, S2=S2,
        K_A=K_A, K_B=K_B, gi16=gi16,
        deg_arr=deg_arr, deg_row=deg_row,
        row_of_node=row_of_node, rank=rank,
        core_of_node=row_of_node // SLICE,
        local_of_node=row_of_node % SLICE,
        idx_key="gi16",
        pad_slots=128 * S2, real_slots=int(len(S_all) / n_cores),
    )


# --------------------------------------------------------------------------
# Bass program
# --------------------------------------------------------------------------
def build_bass(J, K_u, D0, D1, D2, D3, n_cores=N_CORES, bf16_tables=None):
    import concourse.bass as bass
    import concourse.bacc as bacc
    import concourse.mybir as mybir
    import concourse.tile as tile
    from concourse.masks import make_identity

    if bf16_tables is None:
        bf16_tables = BF16_TABLES
    f32 = mybir.dt.float32
    i32 = mybir.dt.int32
    tdt = mybir.dt.bfloat16 if bf16_tables else f32
    S = int(np.sum(K_u))
    off = np.concatenate([[0], np.cumsum(K_u)[:-1]]).astype(np.int64)
    n_pad = J * P * n_cores
    R = n_pad + 1
    rg = [list(range(n_cores))]

    nc = bacc.Bacc("TRN2", target_bir_lowering=False, num_devices=n_cores)

    x_s = nc.dram_tensor("x_s", [J * P, D0], f32, kind="ExternalInput")
    degt = nc.dram_tensor("degt", [P, J], f32, kind="ExternalInput")
    degr = nc.dram_tensor("degr", [1, J * P], f32, kind="ExternalInput")
    gidx = nc.dram_tensor("gidx", [P, S], i32, kind="ExternalInput")
    W1 = nc.dram_tensor("W1", [D0, D1], f32, kind="ExternalInput")
    W2 = nc.dram_tensor("W2", [D1, D2], f32, kind="ExternalInput")
    W3 = nc.dram_tensor("W3", [D2, D3], f32, kind="ExternalInput")
    b1 = nc.dram_tensor("b1", [1, D1], f32, kind="ExternalInput")
    b2 = nc.dram_tensor("b2", [1, D2], f32, kind="ExternalInput")
    b3 = nc.dram_tensor("b3", [1, D3], f32, kind="ExternalInput")
    out = nc.dram_tensor("out", [J * P, D3], f32, kind="ExternalOutput")

    sl1 = nc.dram_tensor("slice1", [J * P, D0], tdt)
    sl2 = nc.dram_tensor("slice2", [J * P, D1], tdt)
    sl3 = nc.dram_tensor("slice3", [J * P, D2], tdt)
    t1 = nc.dram_tensor("table1", [R, D0], tdt, addr_space="Shared")
    t2 = nc.dram_tensor("table2", [R, D1], tdt, addr_space="Shared")
    t3 = nc.dram_tensor("table3", [R, D2], tdt, addr_space="Shared")

    with tile.TileContext(nc) as tc:
        with (
            tc.tile_pool(name="const", bufs=1) as cpool,
            tc.tile_pool(name="gather", bufs=6) as gpool,
            tc.tile_pool(name="work", bufs=4) as wpool,
            tc.tile_pool(name="small", bufs=4) as mpool,
            tc.tile_pool(name="psum", bufs=3, space="PSUM") as ppool,
        ):
            Kmax = max(K_u)

            # ---- constants ----
            ident = cpool.tile([P, P], f32)
            make_identity(nc, ident[:, :])
            gidx_sb = cpool.tile([P, S], i32)
            nc.sync.dma_start(out=gidx_sb[:, :], in_=gidx[:, :])
            W1_sb = cpool.tile([D0, D1], f32)
            nc.sync.dma_start(out=W1_sb[:, :], in_=W1[:, :])
            W2_sb = cpool.tile([D1, D2], f32)
            nc.sync.dma_start(out=W2_sb[:, :], in_=W2[:, :])
            W3_sb = cpool.tile([D2, D3], f32)
            nc.sync.dma_start(out=W3_sb[:, :], in_=W3[:, :])
            b1_sb = cpool.tile([1, D1], f32)
            nc.sync.dma_start(out=b1_sb[:, :], in_=b1[:, :])
            b2_sb = cpool.tile([1, D2], f32)
            nc.sync.dma_start(out=b2_sb[:, :], in_=b2[:, :])
            b3_sb = cpool.tile([1, D3], f32)
            nc.sync.dma_start(out=b3_sb[:, :], in_=b3[:, :])
            ones_row = cpool.tile([1, P], f32)
            nc.gpsimd.memset(ones_row[:, :], 1.0)

            # ---- degree -> dinv, dinv^2, dinv-row ----
            deg_sb = cpool.tile([P, J], f32)
            nc.sync.dma_start(out=deg_sb[:, :], in_=degt[:, :])
            dinv2 = cpool.tile([P, J], f32)
            nc.vector.reciprocal(out=dinv2[:, :], in_=deg_sb[:, :])
            dinv1 = cpool.tile([P, J], f32)
            nc.scalar.activation(
                out=dinv1[:, :], in_=dinv2[:, :],
                func=mybir.ActivationFunctionType.Sqrt,
            )
            degr_sb = cpool.tile([1, J * P], f32)
            nc.sync.dma_start(out=degr_sb[:, :], in_=degr[:, :])
            drow2 = cpool.tile([1, J * P], f32)
            nc.vector.reciprocal(out=drow2[:, :], in_=degr_sb[:, :])
            dinvr = cpool.tile([1, J * P], f32)
            nc.scalar.activation(
                out=dinvr[:, :], in_=drow2[:, :],
                func=mybir.ActivationFunctionType.Sqrt,
            )

            # ---- zero rows of the tables ----
            zt = cpool.tile([1, max(D0, D1, D2)], tdt)
            nc.gpsimd.memset(zt[:, :], 0.0)
            nc.gpsimd.dma_start(out=t1[n_pad : n_pad + 1, :], in_=zt[:1, :D0])
            nc.gpsimd.dma_start(out=t2[n_pad : n_pad + 1, :], in_=zt[:1, :D1])
            nc.gpsimd.dma_start(out=t3[n_pad : n_pad + 1, :], in_=zt[:1, :D2])

            # ---- X̂1 = dinv ⊙ x (own shard) ----
            for jj in range(J):
                xt = wpool.tile([P, D0], f32, tag="xprep")
                nc.sync.dma_start(out=xt[:, :], in_=x_s[jj * P : (jj + 1) * P, :])
                xs = wpool.tile([P, D0], tdt, tag="xprep2")
                nc.vector.tensor_scalar_mul(
                    out=xs[:, :], in0=xt[:, :], scalar1=dinv1[:, jj : jj + 1]
                )
                nc.sync.dma_start(out=sl1[jj * P : (jj + 1) * P, :], in_=xs[:, :])

            if ABLATE != "ag":
                nc.gpsimd.collective_compute(
                    "AllGather", mybir.AluOpType.bypass, replica_groups=rg,
                    ins=[sl1[:, :]], outs=[t1[0:n_pad, :]],
                )

            def layer(table, dst_dram, W_sb, b_sb, Din, Dout, scale_sb, bias_ap,
                      relu, softmax):
                for jj in range(J):
                    K = K_u[jj]
                    o = int(off[jj])
                    G = gpool.tile([P, Kmax, Din], tdt, tag="g")
                    # NOTE: HW indirect DMA only honors per-partition column
                    # offsets ([P,1] -> [P,D]); a 2-D offset AP mis-gathers.
                    for k in range(K if ABLATE != "gather" else 0):
                        nc.gpsimd.indirect_dma_start(
                            out=G[:, k, :],
                            out_offset=None,
                            in_=table[:, :],
                            in_offset=bass.IndirectOffsetOnAxis(
                                ap=gidx_sb[:, o + k : o + k + 1], axis=0
                            ),
                        )
                    # tree reduction over the K slots (into f32 when bf16 tables)
                    if bf16_tables:
                        Hx = gpool.tile([P, (Kmax + 1) // 2, Din], f32, tag="h")
                        k = K
                        if k == 1:
                            nc.vector.tensor_copy(out=Hx[:, 0, :], in_=G[:, 0, :])
                        else:
                            m = k // 2
                            nc.vector.tensor_tensor(
                                out=Hx[:, :m, :], in0=G[:, :m, :],
                                in1=G[:, k - m : k, :], op=mybir.AluOpType.add,
                            )
                            if k - m > m:
                                nc.vector.tensor_copy(
                                    out=Hx[:, m : m + 1, :], in_=G[:, m : m + 1, :]
                                )
                            k -= m
                            while k > 1:
                                m = k // 2
                                nc.vector.tensor_tensor(
                                    out=Hx[:, :m, :], in0=Hx[:, :m, :],
                                    in1=Hx[:, k - m : k, :], op=mybir.AluOpType.add,
                                )
                                k -= m
                        A = Hx[:, 0, :]  # [P, Din] f32
                    else:
                        k = K
                        while k > 1:
                            m = k // 2
                            nc.vector.tensor_tensor(
                                out=G[:, :m, :],
                                in0=G[:, :m, :],
                                in1=G[:, k - m : k, :],
                                op=mybir.AluOpType.add,
                            )
                            k -= m
                        A = G[:, 0, :]  # [P, Din]
                    # per-node normalization
                    nc.vector.tensor_scalar_mul(
                        out=A, in0=A, scalar1=scale_sb[:, jj : jj + 1]
                    )
                    # transpose -> [Din, P]
                    at_ps = ppool.tile([P, P], f32, tag="tpose")
                    nc.tensor.transpose(
                        out=at_ps[:Din, :], in_=A, identity=ident[:, :]
                    )
                    at_sb = wpool.tile([P, P], f32, tag="at")
                    nc.vector.tensor_copy(out=at_sb[:Din, :], in_=at_ps[:Din, :])
                    # dense projection + rank-1 bias
                    z = ppool.tile([P, Dout], f32, tag="z")
                    nc.tensor.matmul(
                        out=z[:, :Dout], lhsT=at_sb[:Din, :], rhs=W_sb[:Din, :Dout],
                        start=True, stop=False,
                    )
                    nc.tensor.matmul(
                        out=z[:, :Dout], lhsT=bias_ap(jj),
                        rhs=b_sb[:1, :Dout], start=False, stop=True,
                    )
                    T = wpool.tile([P, Dout], f32 if softmax else tdt, tag="t")
                    if relu:
                        nc.scalar.activation(
                            out=T[:, :Dout], in_=z[:, :Dout],
                            func=mybir.ActivationFunctionType.Relu,
                        )
                    elif softmax:
                        mneg = mpool.tile([P, 1], f32, tag="mneg")
                        nc.vector.tensor_reduce(
                            out=mneg[:, :], in_=z[:, :Dout],
                            axis=mybir.AxisListType.X, op=mybir.AluOpType.max,
                            negate=True,
                        )
                        nc.scalar.activation(
                            out=T[:, :Dout], in_=z[:, :Dout],
                            func=mybir.ActivationFunctionType.Exp,
                            bias=mneg[:, :1],
                        )
                        ssum = mpool.tile([P, 1], f32, tag="ssum")
                        nc.vector.tensor_reduce(
                            out=ssum[:, :], in_=T[:, :Dout],
                            axis=mybir.AxisListType.X, op=mybir.AluOpType.add,
                        )
                        rec = mpool.tile([P, 1], f32, tag="rec")
                        nc.vector.reciprocal(out=rec[:, :], in_=ssum[:, :])
                        nc.vector.tensor_scalar_mul(
                            out=T[:, :Dout], in0=T[:, :Dout], scalar1=rec[:, :1]
                        )
                    else:
                        nc.vector.tensor_copy(out=T[:, :Dout], in_=z[:, :Dout])
                    nc.sync.dma_start(
                        out=dst_dram[jj * P : (jj + 1) * P, :], in_=T[:, :Dout]
                    )

            dinvr_ap = lambda jj: dinvr[0:1, jj * P : (jj + 1) * P]
            ones_ap = lambda jj: ones_row[0:1, :]

            # layer 1: table1 -> slice2 ; scale dinv^2 ; bias dinv*b1 ; relu
            layer(t1, sl2, W1_sb, b1_sb, D0, D1, dinv2, dinvr_ap, True, False)
            if ABLATE != "ag":
                nc.gpsimd.collective_compute(
                    "AllGather", mybir.AluOpType.bypass, replica_groups=rg,
                    ins=[sl2[:, :]], outs=[t2[0:n_pad, :]],
                )
            # layer 2: no relu
            layer(t2, sl3, W2_sb, b2_sb, D1, D2, dinv2, dinvr_ap, False, False)
            if ABLATE != "ag":
                nc.gpsimd.collective_compute(
                    "AllGather", mybir.AluOpType.bypass, replica_groups=rg,
                    ins=[sl3[:, :]], outs=[t3[0:n_pad, :]],
                )
            # layer 3: scale dinv ; bias 1*b3 ; softmax
            layer(t3, out, W3_sb, b3_sb, D2, D3, dinv1, ones_ap, False, True)

    nc.compile()
    return nc


def build_bass2(J, K_A, K_B, D0, D1, D2, D3, n_cores=N_CORES, bf16_tables=None):
    """Bulk-gather variant: one dma_gather per (block, table-half)."""
    import concourse.bacc as bacc
    import concourse.mybir as mybir
    import concourse.tile as tile
    from concourse.masks import make_identity

    if bf16_tables is None:
        bf16_tables = BF16_TABLES
    f32 = mybir.dt.float32
    i16 = mybir.dt.int16
    tdt = mybir.dt.bfloat16 if bf16_tables else f32
    td3 = f32  # 64-elem bf16 rows would be 128B < dma_gather's 256B granularity
    SLICE = J * P + 1
    R = n_cores * SLICE
    B = 5 * SLICE
    S2 = int(np.sum(K_A) + np.sum(K_B))
    Kmax = max(ka + kb for ka, kb in zip(K_A, K_B))
    off8 = []
    acc = 0
    for jj in range(J):
        off8.append(acc * 8)
        acc += K_A[jj] + K_B[jj]
    rg = [list(range(n_cores))]

    nc = bacc.Bacc("TRN2", target_bir_lowering=False, num_devices=n_cores)

    x_s = nc.dram_tensor("x_s", [J * P, D0], f32, kind="ExternalInput")
    degt = nc.dram_tensor("degt", [P, J], f32, kind="ExternalInput")
    degr = nc.dram_tensor("degr", [1, J * P], f32, kind="ExternalInput")
    gi16 = nc.dram_tensor("gi16", [P, 8 * S2], i16, kind="ExternalInput")
    W1 = nc.dram_tensor("W1", [D0, D1], f32, kind="ExternalInput")
    W2 = nc.dram_tensor("W2", [D1, D2], f32, kind="ExternalInput")
    W3 = nc.dram_tensor("W3", [D2, D3], f32, kind="ExternalInput")
    b1 = nc.dram_tensor("b1", [1, D1], f32, kind="ExternalInput")
    b2 = nc.dram_tensor("b2", [1, D2], f32, kind="ExternalInput")
    b3 = nc.dram_tensor("b3", [1, D3], f32, kind="ExternalInput")
    out = nc.dram_tensor("out", [J * P, D3], f32, kind="ExternalOutput")

    sl1 = nc.dram_tensor("slice1", [SLICE, D0], tdt)
    sl2 = nc.dram_tensor("slice2", [SLICE, D1], tdt)
    sl3 = nc.dram_tensor("slice3", [SLICE, D2], td3)
    t1 = nc.dram_tensor("table1", [R, D0], tdt, addr_space="Shared")
    t2 = nc.dram_tensor("table2", [R, D1], tdt, addr_space="Shared")
    t3 = nc.dram_tensor("table3", [R, D2], td3, addr_space="Shared")

    with tile.TileContext(nc) as tc:
        with (
            tc.tile_pool(name="const", bufs=1) as cpool,
            tc.tile_pool(name="gather", bufs=4) as gpool,
            tc.tile_pool(name="work", bufs=3) as wpool,
            tc.tile_pool(name="small", bufs=4) as mpool,
            tc.tile_pool(name="psum", bufs=2, space="PSUM") as ppool,
        ):
            # ---- constants ----
            ident = cpool.tile([P, P], f32)
            make_identity(nc, ident[:, :])
            gi16_sb = cpool.tile([P, 8 * S2], i16)
            nc.sync.dma_start(out=gi16_sb[:, :], in_=gi16[:, :])
            W1_sb = cpool.tile([D0, D1], f32)
            nc.sync.dma_start(out=W1_sb[:, :], in_=W1[:, :])
            W2_sb = cpool.tile([D1, D2], f32)
            nc.sync.dma_start(out=W2_sb[:, :], in_=W2[:, :])
            W3_sb = cpool.tile([D2, D3], f32)
            nc.sync.dma_start(out=W3_sb[:, :], in_=W3[:, :])
            b1_sb = cpool.tile([1, D1], f32)
            nc.sync.dma_start(out=b1_sb[:, :], in_=b1[:, :])
            b2_sb = cpool.tile([1, D2], f32)
            nc.sync.dma_start(out=b2_sb[:, :], in_=b2[:, :])
            b3_sb = cpool.tile([1, D3], f32)
            nc.sync.dma_start(out=b3_sb[:, :], in_=b3[:, :])
            ones_row = cpool.tile([1, P], f32)
            nc.gpsimd.memset(ones_row[:, :], 1.0)

            # ---- degree -> dinv, dinv^2, dinv-row ----
            deg_sb = cpool.tile([P, J], f32)
            nc.sync.dma_start(out=deg_sb[:, :], in_=degt[:, :])
            dinv2 = cpool.tile([P, J], f32)
            nc.vector.reciprocal(out=dinv2[:, :], in_=deg_sb[:, :])
            dinv1 = cpool.tile([P, J], f32)
            nc.scalar.activation(
                out=dinv1[:, :], in_=dinv2[:, :],
                func=mybir.ActivationFunctionType.Sqrt,
            )
            degr_sb = cpool.tile([1, J * P], f32)
            nc.sync.dma_start(out=degr_sb[:, :], in_=degr[:, :])
            drow2 = cpool.tile([1, J * P], f32)
            nc.vector.reciprocal(out=drow2[:, :], in_=degr_sb[:, :])
            dinvr = cpool.tile([1, J * P], f32)
            nc.scalar.activation(
                out=dinvr[:, :], in_=drow2[:, :],
                func=mybir.ActivationFunctionType.Sqrt,
            )

            # ---- zero row of each slice (pad-gather target; rides the AG) ----
            zt = cpool.tile([1, max(D0, D1)], tdt)
            nc.gpsimd.memset(zt[:, :], 0.0)
            nc.sync.dma_start(out=sl1[J * P : SLICE, :], in_=zt[:1, :D0])
            nc.sync.dma_start(out=sl2[J * P : SLICE, :], in_=zt[:1, :D1])
            zt3 = cpool.tile([1, D2], td3)
            nc.gpsimd.memset(zt3[:, :], 0.0)
            nc.sync.dma_start(out=sl3[J * P : SLICE, :], in_=zt3[:1, :D2])

            # ---- X̂1 = dinv ⊙ x (own shard) ----
            for jj in range(J):
                xt = wpool.tile([P, D0], f32, tag="xprep")
                nc.sync.dma_start(out=xt[:, :], in_=x_s[jj * P : (jj + 1) * P, :])
                xs = wpool.tile([P, D0], tdt, tag="xprep2")
                nc.vector.tensor_scalar_mul(
                    out=xs[:, :], in0=xt[:, :], scalar1=dinv1[:, jj : jj + 1]
                )
                nc.sync.dma_start(out=sl1[jj * P : (jj + 1) * P, :], in_=xs[:, :])

            if ABLATE != "ag":
                nc.gpsimd.collective_compute(
                    "AllGather", mybir.AluOpType.bypass, replica_groups=rg,
                    ins=[sl1[:, :]], outs=[t1[0:R, :]],
                )

            def layer(table, dst_dram, W_sb, b_sb, Din, Dout, scale_sb, bias_ap,
                      relu, softmax, gdt, out_dt):
                for jj in range(J):
                    KA, KB = K_A[jj], K_B[jj]
                    K = KA + KB
                    o8 = off8[jj]
                    G = gpool.tile([P, Kmax, Din], gdt, tag="g")
                    if ABLATE != "gather":
                        if KA:
                            nc.gpsimd.dma_gather(
                                G[:, :KA, :], table[0:B, :],
                                gi16_sb[:, o8 : o8 + 8 * KA],
                                P * KA, P * KA, Din,
                            )
                        if KB:
                            nc.gpsimd.dma_gather(
                                G[:, KA:K, :], table[B:R, :],
                                gi16_sb[:, o8 + 8 * KA : o8 + 8 * K],
                                P * KB, P * KB, Din,
                            )
                    # tree reduction over the K slots (into f32 if gdt is bf16)
                    if gdt != f32:
                        Hx = gpool.tile([P, (Kmax + 1) // 2, Din], f32, tag="h")
                        k = K
                        if k == 1:
                            nc.vector.tensor_copy(out=Hx[:, 0, :], in_=G[:, 0, :])
                        else:
                            m = k // 2
                            nc.vector.tensor_tensor(
                                out=Hx[:, :m, :], in0=G[:, :m, :],
                                in1=G[:, k - m : k, :], op=mybir.AluOpType.add,
                            )
                            if k - m > m:
                                nc.vector.tensor_copy(
                                    out=Hx[:, m : m + 1, :], in_=G[:, m : m + 1, :]
                                )
                            k -= m
                            while k > 1:
                                m = k // 2
                                nc.vector.tensor_tensor(
                                    out=Hx[:, :m, :], in0=Hx[:, :m, :],
                                    in1=Hx[:, k - m : k, :], op=mybir.AluOpType.add,
                                )
                                k -= m
                        A = Hx[:, 0, :]
                    else:
                        k = K
                        while k > 1:
                            m = k // 2
                            nc.vector.tensor_tensor(
                                out=G[:, :m, :], in0=G[:, :m, :],
                                in1=G[:, k - m : k, :], op=mybir.AluOpType.add,
                            )
                            k -= m
                        A = G[:, 0, :]
                    nc.vector.tensor_scalar_mul(
                        out=A, in0=A, scalar1=scale_sb[:, jj : jj + 1]
                    )
                    at_ps = ppool.tile([P, P], f32, tag="tpose")
                    nc.tensor.transpose(
                        out=at_ps[:Din, :], in_=A, identity=ident[:, :]
                    )
                    at_sb = wpool.tile([P, P], f32, tag="at")
                    nc.vector.tensor_copy(out=at_sb[:Din, :], in_=at_ps[:Din, :])
                    z = ppool.tile([P, Dout], f32, tag="z")
                    nc.tensor.matmul(
                        out=z[:, :Dout], lhsT=at_sb[:Din, :], rhs=W_sb[:Din, :Dout],
                        start=True, stop=False,
                    )
                    nc.tensor.matmul(
                        out=z[:, :Dout], lhsT=bias_ap(jj),
                        rhs=b_sb[:1, :Dout], start=False, stop=True,
                    )
                    T = wpool.tile([P, Dout], out_dt, tag="t")
                    if relu:
                        nc.scalar.activation(
                            out=T[:, :Dout], in_=z[:, :Dout],
                            func=mybir.ActivationFunctionType.Relu,
                        )
                    elif softmax:
                        mneg = mpool.tile([P, 1], f32, tag="mneg")
                        nc.vector.tensor_reduce(
                            out=mneg[:, :], in_=z[:, :Dout],
                            axis=mybir.AxisListType.X, op=mybir.AluOpType.max,
                            negate=True,
                        )
                        nc.scalar.activation(
                            out=T[:, :Dout], in_=z[:, :Dout],
                            func=mybir.ActivationFunctionType.Exp,
                            bias=mneg[:, :1],
                        )
                        ssum = mpool.tile([P, 1], f32, tag="ssum")
                        nc.vector.tensor_reduce(
                            out=ssum[:, :], in_=T[:, :Dout],
                            axis=mybir.AxisListType.X, op=mybir.AluOpType.add,
                        )
                        rec = mpool.tile([P, 1], f32, tag="rec")
                        nc.vector.reciprocal(out=rec[:, :], in_=ssum[:, :])
                        nc.vector.tensor_scalar_mul(
                            out=T[:, :Dout], in0=T[:, :Dout], scalar1=rec[:, :1]
                        )
                    else:
                        nc.vector.tensor_copy(out=T[:, :Dout], in_=z[:, :Dout])
                    nc.sync.dma_start(
                        out=dst_dram[jj * P : (jj + 1) * P, :], in_=T[:, :Dout]
                    )

            dinvr_ap = lambda jj: dinvr[0:1, jj * P : (jj + 1) * P]
            ones_ap = lambda jj: ones_row[0:1, :]

            layer(t1, sl2, W1_sb, b1_sb, D0, D1, dinv2, dinvr_ap, True, False,
                  tdt, tdt)
            if ABLATE != "ag":
                nc.gpsimd.collective_compute(
                    "AllGather", mybir.AluOpType.bypass, replica_groups=rg,
                    ins=[sl2[:, :]], outs=[t2[0:R, :]],
                )
            layer(t2, sl3, W2_sb, b2_sb, D1, D2, dinv2, dinvr_ap, False, False,
                  tdt, td3)
            if ABLATE != "ag":
                nc.gpsimd.collective_compute(
                    "AllGather", mybir.AluOpType.bypass, replica_groups=rg,
                    ins=[sl3[:, :]], outs=[t3[0:R, :]],
                )
            layer(t3, out, W3_sb, b3_sb, D2, D3, dinv1, ones_ap, False, True,
                  td3, mybir.dt.float32)

    nc.compile()
    return nc


def make_runner(nc, n_cores=N_CORES):
    """Build the shard_map'd executable once; return (run_fn, time_fn).

    run_fn(in_maps) -> list of per-core output dicts (numpy).
    time_fn(in_maps, iters) -> list of per-iter wall seconds with inputs
    pre-placed on device (H2D excluded).
    """
    import jax
    import numpy as np2
    from jax.sharding import Mesh, PartitionSpec, NamedSharding
    from jax.experimental.shard_map import shard_map
    import concourse.mybir as mybir
    from concourse import bass2jax

    bass2jax.install_neuronx_cc_hook()

    in_names, out_names, out_avals, zero_outs = [], [], [], []
    for alloc in nc.m.functions[0].allocations:
        if not isinstance(alloc, mybir.MemoryLocationSet):
            continue
        name = alloc.memorylocations[0].name
        if alloc.kind == "ExternalInput":
            in_names.append(name)
        elif alloc.kind == "ExternalOutput":
            out_names.append(name)
            shape = tuple(alloc.tensor_shape)
            dtype = mybir.dt.np(alloc.dtype)
            out_avals.append(jax.core.ShapedArray(shape, dtype))
            zero_outs.append(np2.zeros(shape, dtype))
    partition_name = nc.partition_id_tensor.name if nc.partition_id_tensor else None
    if partition_name is not None and partition_name in in_names:
        in_names.remove(partition_name)
    n_params = len(in_names)
    n_outs = len(out_avals)
    all_in_names = in_names + out_names
    if partition_name is not None:
        all_in_names = all_in_names + [partition_name]

    def _body(*args):
        operands = list(args)
        if partition_name is not None:
            operands.append(bass2jax.partition_id_tensor())
        outs = bass2jax._bass_exec_p.bind(
            *operands,
            out_avals=tuple(out_avals),
            in_names=tuple(all_in_names),
            out_names=tuple(out_names),
            lowering_input_output_aliases=(),
            sim_require_finite=True,
            sim_require_nnan=True,
            nc=nc,
        )
        return tuple(outs)

    devices = jax.devices()[:n_cores]
    mesh = Mesh(np2.asarray(devices), ("core",))
    in_specs = (PartitionSpec("core"),) * (n_params + n_outs)
    out_specs = (PartitionSpec("core"),) * n_outs
    donate = tuple(range(n_params, n_params + n_outs))
    sharded = jax.jit(
        shard_map(_body, mesh=mesh, in_specs=in_specs, out_specs=out_specs,
                  check_rep=False),
        donate_argnums=donate, keep_unused=True,
    )
    sh = NamedSharding(mesh, PartitionSpec("core"))

    def _concat_inputs(in_maps):
        return [
            np2.concatenate([np2.asarray(in_maps[c][nm]) for c in range(n_cores)], axis=0)
            for nm in in_names
        ]

    def _zeros():
        return [np2.zeros((n_cores * z.shape[0], *z.shape[1:]), z.dtype)
                for z in zero_outs]

    def run_fn(in_maps):
        out_arrs = sharded(*_concat_inputs(in_maps), *_zeros())
        return [
            {nm: np2.asarray(out_arrs[i]).reshape(n_cores, *out_avals[i].shape)[c]
             for i, nm in enumerate(out_names)}
            for c in range(n_cores)
        ]

    def time_fn(in_maps, iters=5):
        import time as _t
        dev_in = [jax.device_put(a, sh) for a in _concat_inputs(in_maps)]
        for a in dev_in:
            a.block_until_ready()
        times = []
        for _ in range(iters):
            zs = [jax.device_put(z, sh) for z in _zeros()]
            for z in zs:
                z.block_until_ready()
            t0 = _t.time()
            outs = sharded(*dev_in, *zs)
            for o in outs:
                o.block_until_ready()
            times.append(_t.time() - t0)
        return times

    return run_fn, time_fn


# --------------------------------------------------------------------------
# Entry point
# --------------------------------------------------------------------------
def kernel(x, edge_index, W1, b1, W2, b2, W3, b3, _trace=False, _timed=0):
    from concourse.bass_utils import run_bass_kernel_spmd

    x = np.asarray(x, dtype=np.float32)
    W1 = np.asarray(W1, dtype=np.float32)
    W2 = np.asarray(W2, dtype=np.float32)
    W3 = np.asarray(W3, dtype=np.float32)
    b1 = np.asarray(b1, dtype=np.float32)
    b2 = np.asarray(b2, dtype=np.float32)
    b3 = np.asarray(b3, dtype=np.float32)
    n, D0 = x.shape
    D1 = W1.shape[1]
    D2 = W2.shape[1]
    D3 = W3.shape[1]

    pre, nc = build_all(edge_index, n, D0, D1, D2, D3)
    in_maps = shard_inputs(pre, x, W1, b1, W2, b2, W3, b3)

    if _timed:
        run_fn, time_fn = make_runner(nc)
        results = run_fn(in_maps)
        times = time_fn(in_maps, _timed)
    else:
        res = run_bass_kernel_spmd(
            nc, in_maps, core_ids=list(range(N_CORES)), trace=_trace
        )
        results = res.results
        times = None

    full = unshard(pre, results)
    if _timed:
        return full.astype(np.float32), times
    return full.astype(np.float32)


def build_all(edge_index, n, D0, D1, D2, D3):
    """Preprocess + build the Bass program per GATHER_MODE."""
    if GATHER_MODE == "bulk":
        pre = preprocess2(edge_index, n)
        nc = build_bass2(pre["J"], pre["K_A"], pre["K_B"], D0, D1, D2, D3,
                         bf16_tables=BF16_TABLES)
    else:
        pre = preprocess(edge_index, n)
        nc = build_bass(pre["J"], pre["K_u"], D0, D1, D2, D3,
                        bf16_tables=BF16_TABLES)
    return pre, nc


def shard_inputs(pre, x, W1, b1, W2, b2, W3, b3):
    J = pre["J"]
    D0 = x.shape[1]
    D1, D2, D3 = W1.shape[1], W2.shape[1], W3.shape[1]
    x_sh = np.zeros((N_CORES, J * P, D0), dtype=np.float32)
    x_sh[pre["core_of_node"], pre["local_of_node"]] = x
    idx_key = pre["idx_key"]
    idx_arr = pre["gi16"] if idx_key == "gi16" else pre["idx"]
    in_maps = []
    for c in range(N_CORES):
        in_maps.append(
            {
                "x_s": np.ascontiguousarray(x_sh[c]),
                "degt": np.ascontiguousarray(pre["deg_arr"][c]),
                "degr": np.ascontiguousarray(pre["deg_row"][c]),
                idx_key: np.ascontiguousarray(idx_arr[c]),
                "W1": np.asarray(W1, np.float32),
                "W2": np.asarray(W2, np.float32),
                "W3": np.asarray(W3, np.float32),
                "b1": np.asarray(b1, np.float32).reshape(1, D1),
                "b2": np.asarray(b2, np.float32).reshape(1, D2),
                "b3": np.asarray(b3, np.float32).reshape(1, D3),
            }
        )
    return in_maps


def unshard(pre, results):
    out_all = np.stack([results[c]["out"] for c in range(N_CORES)])
    return out_all[pre["core_of_node"], pre["local_of_node"]]



# revision 6
# speedup vs baseline: 1.3767x; 1.3767x over previous
"""Trainium2 Bass kernel for a 3-layer GCN (ExtendedGCN).

Math (per reference):
    agg(F) = D^-1/2 (A + I) D^-1/2 F      with deg = in-degree + 1
    Z1 = agg(x) @ W1 + b1 ; H1 = relu(Z1)
    Z2 = agg(H1) @ W2 + b2
    Z3 = agg(H2=Z2) @ W3 + b3 ; out = softmax(Z3, axis=1)
(aggregate-then-project is exact: message passing commutes with the
right-multiplication by W).

Distribution: nodes are partitioned across 8 cores (dst-owner edge split).
Each layer, every core computes its own node rows, then the scaled feature
table X̂ = dinv ⊙ H is AllGathered so every core can gather arbitrary source
rows locally.  Per-node contributor lists (in-neighbors + self-loop) are
precomputed on the host as table-row indices, grouped per 128-node block so a
single indirect DMA gathers a [128, K, D] tile and a short in-place tree of
vector adds produces the aggregate.

Folding of the symmetric normalization: with X̂_l = dinv ⊙ H_l as the gather
table, S = plain sum of gathered rows (self-loop included as an ordinary
slot), the next table is directly
    X̂_{l+1} = relu?( (dinv^2 ⊙ S) @ W_l + dinv*b_l )
and the final logits are Z3 = (dinv ⊙ S3) @ W3 + b3.
"""

import sys

sys.path.insert(0, "/opt/trn_rl_repo")

import numpy as np

N_CORES = 8
P = 128  # partitions / block size
BF16_TABLES = False  # bf16 feature tables: halves gather + all-gather traffic
GATHER_MODE = "indirect"  # "indirect" (per-column indirect DMA) | "bulk" (dma_gather)
ABLATE = ""  # dev-only: "ag" skips collectives, "gather" skips table gathers
REPS = 1  # dev-only: repeat the whole computation REPS times inside one program


# --------------------------------------------------------------------------
# Host-side graph preprocessing (integer index work only)
# --------------------------------------------------------------------------
def preprocess(edge_index, n_nodes, n_cores=N_CORES):
    src = np.asarray(edge_index[0]).astype(np.int64)
    dst = np.asarray(edge_index[1]).astype(np.int64)

    deg = np.bincount(dst, minlength=n_nodes).astype(np.int64) + 1  # + self

    # order nodes by degree (desc) so blocks have uniform slot counts
    order = np.argsort(-deg, kind="stable")  # sorted position k -> node id
    chunk = P * n_cores
    n_pad = ((n_nodes + chunk - 1) // chunk) * chunk
    J = n_pad // chunk  # blocks per core
    ZROW = n_pad  # index of the all-zero table row

    k = np.arange(n_pad)
    g = k // P  # global block
    core_of_k = g % n_cores
    jj_of_k = g // n_cores
    row_of_k = core_of_k * (J * P) + jj_of_k * P + (k % P)

    rank = np.empty(n_nodes, dtype=np.int64)
    rank[order] = np.arange(n_nodes)
    row_of_node = row_of_k[rank]  # node id -> table row

    deg_sorted = deg[order]  # desc
    K_u = []
    for jj in range(J):
        k0 = jj * chunk
        K_u.append(int(deg_sorted[k0]) if k0 < n_nodes else 1)
    S = int(np.sum(K_u))
    off = np.concatenate([[0], np.cumsum(K_u)[:-1]]).astype(np.int64)

    # slot lists: idx[core, p, off[jj]+s] = table row of s-th contributor
    idx = np.full((n_cores, P, S), ZROW, dtype=np.int32)

    # self-loop entries (slot 0) for real nodes
    kr = rank  # k of each real node
    idx[core_of_k[kr], kr % P, off[jj_of_k[kr]]] = row_of_node.astype(np.int32)

    # edge entries, slots 1..cnt
    er = rank[dst]  # sorted-position of each edge's dst
    eorder = np.argsort(er, kind="stable")
    er_s = er[eorder]
    src_rows = row_of_node[src[eorder]].astype(np.int32)
    cnt = np.bincount(er_s, minlength=n_pad)
    start = np.concatenate([[0], np.cumsum(cnt)[:-1]])
    slot = np.arange(len(er_s)) - start[er_s] + 1
    col = off[jj_of_k[er_s]] + slot
    idx[core_of_k[er_s], er_s % P, col] = src_rows

    # per-core degree array [P, J] (deg of local node (jj,p) at [p,jj])
    deg_by_row = np.ones(n_pad, dtype=np.float32)
    deg_by_row[row_of_node] = deg.astype(np.float32)
    deg_arr = deg_by_row.reshape(n_cores, J, P).transpose(0, 2, 1).copy()
    # row layout [1, J*P] (deg of local node (jj,p) at [0, jj*P+p])
    deg_row = deg_by_row.reshape(n_cores, 1, J * P).copy()

    return dict(
        n_pad=n_pad,
        J=J,
        S=S,
        K_u=K_u,
        off=off,
        idx=idx,
        deg_arr=deg_arr,
        deg_row=deg_row,
        row_of_node=row_of_node,
        core_of_node=row_of_node // (J * P),
        local_of_node=row_of_node % (J * P),
        idx_key="gidx",
    )


def preprocess2(edge_index, n_nodes, n_cores=N_CORES):
    """Host preprocessing for the bulk dma_gather path.

    Table layout: 8 per-core slices of SLICE = J*128+1 rows each; the last
    row of every slice is all-zeros (gather target for padding).  int16
    index limit: the lo half = first 5 slices (rows [0, 5*SLICE)), hi half =
    remaining 3 slices; per (block, half) the per-node slot lists are padded
    to the block's max count, indices stored half-relative in the wrapped
    [16]-partition int16 layout dma_gather expects.
    """
    src = np.asarray(edge_index[0]).astype(np.int64)
    dst = np.asarray(edge_index[1]).astype(np.int64)
    n = n_nodes
    deg = np.bincount(dst, minlength=n).astype(np.int64) + 1

    chunk = P * n_cores
    n_pad = ((n + chunk - 1) // chunk) * chunk
    J = n_pad // chunk
    SLICE = J * P + 1
    R = n_cores * SLICE
    N_LO = 5
    B = N_LO * SLICE  # lo/hi boundary row
    assert B - 1 <= 32767 and R - B - 1 <= 32767

    S_all = np.concatenate([src, np.arange(n)])
    D_all = np.concatenate([dst, np.arange(n)])

    def lo_of_rank(r):
        return (r // P) % n_cores < N_LO

    order = np.argsort(-deg, kind="stable")
    for _ in range(2):
        rank = np.empty(n, np.int64)
        rank[order] = np.arange(n)
        is_lo = lo_of_rank(rank[S_all])
        lo = np.zeros(n, np.int64)
        np.add.at(lo, D_all, is_lo)
        hi = deg - lo
        order = np.lexsort((-hi, -lo))
    rank = np.empty(n, np.int64)
    rank[order] = np.arange(n)
    is_lo = lo_of_rank(rank[S_all])
    lo = np.zeros(n, np.int64)
    np.add.at(lo, D_all, is_lo)
    hi = deg - lo

    # rank -> (core, jj, p) -> table row
    def row_of_rank(r):
        g = r // P
        return (g % n_cores) * SLICE + (g // n_cores) * P + (r % P)

    row_of_node = row_of_rank(rank)

    # uniform per-block-index slot counts (max over the 8 cores)
    lo_pad = np.zeros(n_pad, np.int64)
    lo_pad[rank] = lo
    hi_pad = np.zeros(n_pad, np.int64)
    hi_pad[rank] = hi
    K_A = [int(lo_pad[jj * chunk : (jj + 1) * chunk].max()) for jj in range(J)]
    K_B = [int(hi_pad[jj * chunk : (jj + 1) * chunk].max()) for jj in range(J)]
    S2 = int(np.sum(K_A) + np.sum(K_B))

    # gi16 [cores, 128, 8*S2] prefilled with the zero-row relative index
    ZREL = J * P  # 6272 both halves (core0-zero for lo, core(N_LO)-zero for hi)
    gi16 = np.full((n_cores, 16, 8 * S2), ZREL, dtype=np.int16)
    col0 = np.zeros((J, 2), np.int64)  # column offset (in slot cols) per (jj, half)
    acc = 0
    for jj in range(J):
        col0[jj, 0] = acc
        acc += K_A[jj]
        col0[jj, 1] = acc
        acc += K_B[jj]

    er = rank[D_all]  # dst rank of each (edge incl self)
    src_row = row_of_node[S_all]
    for half in (0, 1):
        sel = np.where(is_lo if half == 0 else ~is_lo)[0]
        ers = er[sel]
        eorder = np.argsort(ers, kind="stable")
        ers = ers[eorder]
        rows = src_row[sel][eorder] - (0 if half == 0 else B)
        cnt = np.bincount(ers, minlength=n_pad)
        start = np.concatenate([[0], np.cumsum(cnt)[:-1]])
        s = np.arange(len(ers)) - start[ers]
        g = ers // P
        c = g % n_cores
        jjv = g // n_cores
        p = ers % P
        j = s * P + p  # index position within the call
        col = col0[jjv, half] * 8 + j // 16
        gi16[c, j % 16, col] = rows.astype(np.int16)
    gi16 = np.tile(gi16, (1, 8, 1))  # replicate 16-row wrap to 128 partitions

    deg_by_rank = np.ones(n_pad, dtype=np.float32)
    deg_by_rank[rank] = deg.astype(np.float32)
    deg_arr = deg_by_rank.reshape(J, n_cores, P).transpose(1, 2, 0).copy()
    deg_row = deg_by_rank.reshape(J, n_cores, P).transpose(1, 0, 2).reshape(
        n_cores, 1, J * P
    ).copy()

    return dict(
        n_pad=n_pad, J=J, SLICE=SLICE, R=R, B=B, S2=S2,
        K_A=K_A, K_B=K_B, gi16=gi16,
        deg_arr=deg_arr, deg_row=deg_row,
        row_of_node=row_of_node, rank=rank,
        core_of_node=row_of_node // SLICE,
        local_of_node=row_of_node % SLICE,
        idx_key="gi16",
        pad_slots=128 * S2, real_slots=int(len(S_all) / n_cores),
    )


# --------------------------------------------------------------------------
# Bass program
# --------------------------------------------------------------------------
def build_bass(J, K_u, D0, D1, D2, D3, n_cores=N_CORES, bf16_tables=None):
    import concourse.bass as bass
    import concourse.bacc as bacc
    import concourse.mybir as mybir
    import concourse.tile as tile
    from concourse.masks import make_identity

    if bf16_tables is None:
        bf16_tables = BF16_TABLES
    f32 = mybir.dt.float32
    i32 = mybir.dt.int32
    tdt = mybir.dt.bfloat16 if bf16_tables else f32
    S = int(np.sum(K_u))
    off = np.concatenate([[0], np.cumsum(K_u)[:-1]]).astype(np.int64)
    n_pad = J * P * n_cores
    R = n_pad + 1
    rg = [list(range(n_cores))]

    nc = bacc.Bacc("TRN2", target_bir_lowering=False, num_devices=n_cores)

    x_s = nc.dram_tensor("x_s", [J * P, D0], f32, kind="ExternalInput")
    degt = nc.dram_tensor("degt", [P, J], f32, kind="ExternalInput")
    degr = nc.dram_tensor("degr", [1, J * P], f32, kind="ExternalInput")
    gidx = nc.dram_tensor("gidx", [P, S], i32, kind="ExternalInput")
    W1 = nc.dram_tensor("W1", [D0, D1], f32, kind="ExternalInput")
    W2 = nc.dram_tensor("W2", [D1, D2], f32, kind="ExternalInput")
    W3 = nc.dram_tensor("W3", [D2, D3], f32, kind="ExternalInput")
    b1 = nc.dram_tensor("b1", [1, D1], f32, kind="ExternalInput")
    b2 = nc.dram_tensor("b2", [1, D2], f32, kind="ExternalInput")
    b3 = nc.dram_tensor("b3", [1, D3], f32, kind="ExternalInput")
    out = nc.dram_tensor("out", [J * P, D3], f32, kind="ExternalOutput")

    sl1 = nc.dram_tensor("slice1", [J * P, D0], tdt)
    sl2 = nc.dram_tensor("slice2", [J * P, D1], tdt)
    sl3 = nc.dram_tensor("slice3", [J * P, D2], tdt)
    t1 = nc.dram_tensor("table1", [R, D0], tdt, addr_space="Shared")
    t2 = nc.dram_tensor("table2", [R, D1], tdt, addr_space="Shared")
    t3 = nc.dram_tensor("table3", [R, D2], tdt, addr_space="Shared")

    with tile.TileContext(nc) as tc:
        with (
            tc.tile_pool(name="const", bufs=1) as cpool,
            tc.tile_pool(name="gather", bufs=6) as gpool,
            tc.tile_pool(name="work", bufs=4) as wpool,
            tc.tile_pool(name="small", bufs=4) as mpool,
            tc.tile_pool(name="psum", bufs=3, space="PSUM") as ppool,
        ):
            Kmax = max(K_u)

            # ---- constants ----
            ident = cpool.tile([P, P], f32)
            make_identity(nc, ident[:, :])
            gidx_sb = cpool.tile([P, S], i32)
            nc.sync.dma_start(out=gidx_sb[:, :], in_=gidx[:, :])
            W1_sb = cpool.tile([D0, D1], f32)
            nc.sync.dma_start(out=W1_sb[:, :], in_=W1[:, :])
            W2_sb = cpool.tile([D1, D2], f32)
            nc.sync.dma_start(out=W2_sb[:, :], in_=W2[:, :])
            W3_sb = cpool.tile([D2, D3], f32)
            nc.sync.dma_start(out=W3_sb[:, :], in_=W3[:, :])
            b1_sb = cpool.tile([1, D1], f32)
            nc.sync.dma_start(out=b1_sb[:, :], in_=b1[:, :])
            b2_sb = cpool.tile([1, D2], f32)
            nc.sync.dma_start(out=b2_sb[:, :], in_=b2[:, :])
            b3_sb = cpool.tile([1, D3], f32)
            nc.sync.dma_start(out=b3_sb[:, :], in_=b3[:, :])
            ones_row = cpool.tile([1, P], f32)
            nc.gpsimd.memset(ones_row[:, :], 1.0)

            # ---- degree -> dinv, dinv^2, dinv-row ----
            deg_sb = cpool.tile([P, J], f32)
            nc.sync.dma_start(out=deg_sb[:, :], in_=degt[:, :])
            dinv2 = cpool.tile([P, J], f32)
            nc.vector.reciprocal(out=dinv2[:, :], in_=deg_sb[:, :])
            dinv1 = cpool.tile([P, J], f32)
            nc.scalar.activation(
                out=dinv1[:, :], in_=dinv2[:, :],
                func=mybir.ActivationFunctionType.Sqrt,
            )
            degr_sb = cpool.tile([1, J * P], f32)
            nc.sync.dma_start(out=degr_sb[:, :], in_=degr[:, :])
            drow2 = cpool.tile([1, J * P], f32)
            nc.vector.reciprocal(out=drow2[:, :], in_=degr_sb[:, :])
            dinvr = cpool.tile([1, J * P], f32)
            nc.scalar.activation(
                out=dinvr[:, :], in_=drow2[:, :],
                func=mybir.ActivationFunctionType.Sqrt,
            )

            # ---- zero rows of the tables ----
            zt = cpool.tile([1, max(D0, D1, D2)], tdt)
            nc.gpsimd.memset(zt[:, :], 0.0)
            nc.gpsimd.dma_start(out=t1[n_pad : n_pad + 1, :], in_=zt[:1, :D0])
            nc.gpsimd.dma_start(out=t2[n_pad : n_pad + 1, :], in_=zt[:1, :D1])
            nc.gpsimd.dma_start(out=t3[n_pad : n_pad + 1, :], in_=zt[:1, :D2])

            def prep_and_ag1():
                # ---- X̂1 = dinv ⊙ x (own shard) ----
                for jj in range(J):
                    xt = wpool.tile([P, D0], f32, tag="xprep")
                    nc.sync.dma_start(out=xt[:, :], in_=x_s[jj * P : (jj + 1) * P, :])
                    xs = wpool.tile([P, D0], tdt, tag="xprep2")
                    nc.vector.tensor_scalar_mul(
                        out=xs[:, :], in0=xt[:, :], scalar1=dinv1[:, jj : jj + 1]
                    )
                    nc.sync.dma_start(out=sl1[jj * P : (jj + 1) * P, :], in_=xs[:, :])

                if ABLATE != "ag":
                    nc.gpsimd.collective_compute(
                        "AllGather", mybir.AluOpType.bypass, replica_groups=rg,
                        ins=[sl1[:, :]], outs=[t1[0:n_pad, :]],
                    )

            def layer(table, dst_dram, W_sb, b_sb, Din, Dout, scale_sb, bias_ap,
                      relu, softmax):
                for jj in range(J):
                    K = K_u[jj]
                    o = int(off[jj])
                    G = gpool.tile([P, Kmax, Din], tdt, tag="g")
                    # NOTE: HW indirect DMA only honors per-partition column
                    # offsets ([P,1] -> [P,D]); a 2-D offset AP mis-gathers.
                    for k in range(K if ABLATE != "gather" else 0):
                        nc.gpsimd.indirect_dma_start(
                            out=G[:, k, :],
                            out_offset=None,
                            in_=table[:, :],
                            in_offset=bass.IndirectOffsetOnAxis(
                                ap=gidx_sb[:, o + k : o + k + 1], axis=0
                            ),
                        )
                    # tree reduction over the K slots (into f32 when bf16 tables)
                    if bf16_tables:
                        Hx = gpool.tile([P, (Kmax + 1) // 2, Din], f32, tag="h")
                        k = K
                        if k == 1:
                            nc.vector.tensor_copy(out=Hx[:, 0, :], in_=G[:, 0, :])
                        else:
                            m = k // 2
                            nc.vector.tensor_tensor(
                                out=Hx[:, :m, :], in0=G[:, :m, :],
                                in1=G[:, k - m : k, :], op=mybir.AluOpType.add,
                            )
                            if k - m > m:
                                nc.vector.tensor_copy(
                                    out=Hx[:, m : m + 1, :], in_=G[:, m : m + 1, :]
                                )
                            k -= m
                            while k > 1:
                                m = k // 2
                                nc.vector.tensor_tensor(
                                    out=Hx[:, :m, :], in0=Hx[:, :m, :],
                                    in1=Hx[:, k - m : k, :], op=mybir.AluOpType.add,
                                )
                                k -= m
                        A = Hx[:, 0, :]  # [P, Din] f32
                    else:
                        k = K
                        while k > 1:
                            m = k // 2
                            nc.vector.tensor_tensor(
                                out=G[:, :m, :],
                                in0=G[:, :m, :],
                                in1=G[:, k - m : k, :],
                                op=mybir.AluOpType.add,
                            )
                            k -= m
                        A = G[:, 0, :]  # [P, Din]
                    # per-node normalization
                    nc.vector.tensor_scalar_mul(
                        out=A, in0=A, scalar1=scale_sb[:, jj : jj + 1]
                    )
                    # transpose -> [Din, P]
                    at_ps = ppool.tile([P, P], f32, tag="tpose")
                    nc.tensor.transpose(
                        out=at_ps[:Din, :], in_=A, identity=ident[:, :]
                    )
                    at_sb = wpool.tile([P, P], f32, tag="at")
                    nc.vector.tensor_copy(out=at_sb[:Din, :], in_=at_ps[:Din, :])
                    # dense projection + rank-1 bias
                    z = ppool.tile([P, Dout], f32, tag="z")
                    nc.tensor.matmul(
                        out=z[:, :Dout], lhsT=at_sb[:Din, :], rhs=W_sb[:Din, :Dout],
                        start=True, stop=False,
                    )
                    nc.tensor.matmul(
                        out=z[:, :Dout], lhsT=bias_ap(jj),
                        rhs=b_sb[:1, :Dout], start=False, stop=True,
                    )
                    T = wpool.tile([P, Dout], f32 if softmax else tdt, tag="t")
                    if relu:
                        nc.scalar.activation(
                            out=T[:, :Dout], in_=z[:, :Dout],
                            func=mybir.ActivationFunctionType.Relu,
                        )
                    elif softmax:
                        mneg = mpool.tile([P, 1], f32, tag="mneg")
                        nc.vector.tensor_reduce(
                            out=mneg[:, :], in_=z[:, :Dout],
                            axis=mybir.AxisListType.X, op=mybir.AluOpType.max,
                            negate=True,
                        )
                        nc.scalar.activation(
                            out=T[:, :Dout], in_=z[:, :Dout],
                            func=mybir.ActivationFunctionType.Exp,
                            bias=mneg[:, :1],
                        )
                        ssum = mpool.tile([P, 1], f32, tag="ssum")
                        nc.vector.tensor_reduce(
                            out=ssum[:, :], in_=T[:, :Dout],
                            axis=mybir.AxisListType.X, op=mybir.AluOpType.add,
                        )
                        rec = mpool.tile([P, 1], f32, tag="rec")
                        nc.vector.reciprocal(out=rec[:, :], in_=ssum[:, :])
                        nc.vector.tensor_scalar_mul(
                            out=T[:, :Dout], in0=T[:, :Dout], scalar1=rec[:, :1]
                        )
                    else:
                        nc.vector.tensor_copy(out=T[:, :Dout], in_=z[:, :Dout])
                    nc.sync.dma_start(
                        out=dst_dram[jj * P : (jj + 1) * P, :], in_=T[:, :Dout]
                    )

            dinvr_ap = lambda jj: dinvr[0:1, jj * P : (jj + 1) * P]
            ones_ap = lambda jj: ones_row[0:1, :]

            for _rep in range(REPS):
                prep_and_ag1()
                # layer 1: table1 -> slice2 ; scale dinv^2 ; bias dinv*b1 ; relu
                layer(t1, sl2, W1_sb, b1_sb, D0, D1, dinv2, dinvr_ap, True, False)
                if ABLATE != "ag":
                    nc.gpsimd.collective_compute(
                        "AllGather", mybir.AluOpType.bypass, replica_groups=rg,
                        ins=[sl2[:, :]], outs=[t2[0:n_pad, :]],
                    )
                # layer 2: no relu
                layer(t2, sl3, W2_sb, b2_sb, D1, D2, dinv2, dinvr_ap, False, False)
                if ABLATE != "ag":
                    nc.gpsimd.collective_compute(
                        "AllGather", mybir.AluOpType.bypass, replica_groups=rg,
                        ins=[sl3[:, :]], outs=[t3[0:n_pad, :]],
                    )
                # layer 3: scale dinv ; bias 1*b3 ; softmax
                layer(t3, out, W3_sb, b3_sb, D2, D3, dinv1, ones_ap, False, True)

    nc.compile()
    return nc


def build_bass2(J, K_A, K_B, D0, D1, D2, D3, n_cores=N_CORES, bf16_tables=None):
    """Bulk-gather variant: one dma_gather per (block, table-half)."""
    import concourse.bacc as bacc
    import concourse.mybir as mybir
    import concourse.tile as tile
    from concourse.masks import make_identity

    if bf16_tables is None:
        bf16_tables = BF16_TABLES
    f32 = mybir.dt.float32
    i16 = mybir.dt.int16
    tdt = mybir.dt.bfloat16 if bf16_tables else f32
    td3 = f32  # 64-elem bf16 rows would be 128B < dma_gather's 256B granularity
    SLICE = J * P + 1
    R = n_cores * SLICE
    B = 5 * SLICE
    S2 = int(np.sum(K_A) + np.sum(K_B))
    Kmax = max(ka + kb for ka, kb in zip(K_A, K_B))
    off8 = []
    acc = 0
    for jj in range(J):
        off8.append(acc * 8)
        acc += K_A[jj] + K_B[jj]
    rg = [list(range(n_cores))]

    nc = bacc.Bacc("TRN2", target_bir_lowering=False, num_devices=n_cores)

    x_s = nc.dram_tensor("x_s", [J * P, D0], f32, kind="ExternalInput")
    degt = nc.dram_tensor("degt", [P, J], f32, kind="ExternalInput")
    degr = nc.dram_tensor("degr", [1, J * P], f32, kind="ExternalInput")
    gi16 = nc.dram_tensor("gi16", [P, 8 * S2], i16, kind="ExternalInput")
    W1 = nc.dram_tensor("W1", [D0, D1], f32, kind="ExternalInput")
    W2 = nc.dram_tensor("W2", [D1, D2], f32, kind="ExternalInput")
    W3 = nc.dram_tensor("W3", [D2, D3], f32, kind="ExternalInput")
    b1 = nc.dram_tensor("b1", [1, D1], f32, kind="ExternalInput")
    b2 = nc.dram_tensor("b2", [1, D2], f32, kind="ExternalInput")
    b3 = nc.dram_tensor("b3", [1, D3], f32, kind="ExternalInput")
    out = nc.dram_tensor("out", [J * P, D3], f32, kind="ExternalOutput")

    sl1 = nc.dram_tensor("slice1", [SLICE, D0], tdt)
    sl2 = nc.dram_tensor("slice2", [SLICE, D1], tdt)
    sl3 = nc.dram_tensor("slice3", [SLICE, D2], td3)
    t1 = nc.dram_tensor("table1", [R, D0], tdt, addr_space="Shared")
    t2 = nc.dram_tensor("table2", [R, D1], tdt, addr_space="Shared")
    t3 = nc.dram_tensor("table3", [R, D2], td3, addr_space="Shared")

    with tile.TileContext(nc) as tc:
        with (
            tc.tile_pool(name="const", bufs=1) as cpool,
            tc.tile_pool(name="gather", bufs=4) as gpool,
            tc.tile_pool(name="work", bufs=3) as wpool,
            tc.tile_pool(name="small", bufs=4) as mpool,
            tc.tile_pool(name="psum", bufs=2, space="PSUM") as ppool,
        ):
            # ---- constants ----
            ident = cpool.tile([P, P], f32)
            make_identity(nc, ident[:, :])
            gi16_sb = cpool.tile([P, 8 * S2], i16)
            nc.sync.dma_start(out=gi16_sb[:, :], in_=gi16[:, :])
            W1_sb = cpool.tile([D0, D1], f32)
            nc.sync.dma_start(out=W1_sb[:, :], in_=W1[:, :])
            W2_sb = cpool.tile([D1, D2], f32)
            nc.sync.dma_start(out=W2_sb[:, :], in_=W2[:, :])
            W3_sb = cpool.tile([D2, D3], f32)
            nc.sync.dma_start(out=W3_sb[:, :], in_=W3[:, :])
            b1_sb = cpool.tile([1, D1], f32)
            nc.sync.dma_start(out=b1_sb[:, :], in_=b1[:, :])
            b2_sb = cpool.tile([1, D2], f32)
            nc.sync.dma_start(out=b2_sb[:, :], in_=b2[:, :])
            b3_sb = cpool.tile([1, D3], f32)
            nc.sync.dma_start(out=b3_sb[:, :], in_=b3[:, :])
            ones_row = cpool.tile([1, P], f32)
            nc.gpsimd.memset(ones_row[:, :], 1.0)

            # ---- degree -> dinv, dinv^2, dinv-row ----
            deg_sb = cpool.tile([P, J], f32)
            nc.sync.dma_start(out=deg_sb[:, :], in_=degt[:, :])
            dinv2 = cpool.tile([P, J], f32)
            nc.vector.reciprocal(out=dinv2[:, :], in_=deg_sb[:, :])
            dinv1 = cpool.tile([P, J], f32)
            nc.scalar.activation(
                out=dinv1[:, :], in_=dinv2[:, :],
                func=mybir.ActivationFunctionType.Sqrt,
            )
            degr_sb = cpool.tile([1, J * P], f32)
            nc.sync.dma_start(out=degr_sb[:, :], in_=degr[:, :])
            drow2 = cpool.tile([1, J * P], f32)
            nc.vector.reciprocal(out=drow2[:, :], in_=degr_sb[:, :])
            dinvr = cpool.tile([1, J * P], f32)
            nc.scalar.activation(
                out=dinvr[:, :], in_=drow2[:, :],
                func=mybir.ActivationFunctionType.Sqrt,
            )

            # ---- zero row of each slice (pad-gather target; rides the AG) ----
            zt = cpool.tile([1, max(D0, D1)], tdt)
            nc.gpsimd.memset(zt[:, :], 0.0)
            nc.sync.dma_start(out=sl1[J * P : SLICE, :], in_=zt[:1, :D0])
            nc.sync.dma_start(out=sl2[J * P : SLICE, :], in_=zt[:1, :D1])
            zt3 = cpool.tile([1, D2], td3)
            nc.gpsimd.memset(zt3[:, :], 0.0)
            nc.sync.dma_start(out=sl3[J * P : SLICE, :], in_=zt3[:1, :D2])

            def prep_and_ag1():
                # ---- X̂1 = dinv ⊙ x (own shard) ----
                for jj in range(J):
                    xt = wpool.tile([P, D0], f32, tag="xprep")
                    nc.sync.dma_start(out=xt[:, :], in_=x_s[jj * P : (jj + 1) * P, :])
                    xs = wpool.tile([P, D0], tdt, tag="xprep2")
                    nc.vector.tensor_scalar_mul(
                        out=xs[:, :], in0=xt[:, :], scalar1=dinv1[:, jj : jj + 1]
                    )
                    nc.sync.dma_start(out=sl1[jj * P : (jj + 1) * P, :], in_=xs[:, :])

                if ABLATE != "ag":
                    nc.gpsimd.collective_compute(
                        "AllGather", mybir.AluOpType.bypass, replica_groups=rg,
                        ins=[sl1[:, :]], outs=[t1[0:R, :]],
                    )

            def layer(table, dst_dram, W_sb, b_sb, Din, Dout, scale_sb, bias_ap,
                      relu, softmax, gdt, out_dt):
                for jj in range(J):
                    KA, KB = K_A[jj], K_B[jj]
                    K = KA + KB
                    o8 = off8[jj]
                    G = gpool.tile([P, Kmax, Din], gdt, tag="g")
                    if ABLATE != "gather":
                        if KA:
                            nc.gpsimd.dma_gather(
                                G[:, :KA, :], table[0:B, :],
                                gi16_sb[:, o8 : o8 + 8 * KA],
                                P * KA, P * KA, Din,
                            )
                        if KB:
                            nc.gpsimd.dma_gather(
                                G[:, KA:K, :], table[B:R, :],
                                gi16_sb[:, o8 + 8 * KA : o8 + 8 * K],
                                P * KB, P * KB, Din,
                            )
                    # tree reduction over the K slots (into f32 if gdt is bf16)
                    if gdt != f32:
                        Hx = gpool.tile([P, (Kmax + 1) // 2, Din], f32, tag="h")
                        k = K
                        if k == 1:
                            nc.vector.tensor_copy(out=Hx[:, 0, :], in_=G[:, 0, :])
                        else:
                            m = k // 2
                            nc.vector.tensor_tensor(
                                out=Hx[:, :m, :], in0=G[:, :m, :],
                                in1=G[:, k - m : k, :], op=mybir.AluOpType.add,
                            )
                            if k - m > m:
                                nc.vector.tensor_copy(
                                    out=Hx[:, m : m + 1, :], in_=G[:, m : m + 1, :]
                                )
                            k -= m
                            while k > 1:
                                m = k // 2
                                nc.vector.tensor_tensor(
                                    out=Hx[:, :m, :], in0=Hx[:, :m, :],
                                    in1=Hx[:, k - m : k, :], op=mybir.AluOpType.add,
                                )
                                k -= m
                        A = Hx[:, 0, :]
                    else:
                        k = K
                        while k > 1:
                            m = k // 2
                            nc.vector.tensor_tensor(
                                out=G[:, :m, :], in0=G[:, :m, :],
                                in1=G[:, k - m : k, :], op=mybir.AluOpType.add,
                            )
                            k -= m
                        A = G[:, 0, :]
                    nc.vector.tensor_scalar_mul(
                        out=A, in0=A, scalar1=scale_sb[:, jj : jj + 1]
                    )
                    at_ps = ppool.tile([P, P], f32, tag="tpose")
                    nc.tensor.transpose(
                        out=at_ps[:Din, :], in_=A, identity=ident[:, :]
                    )
                    at_sb = wpool.tile([P, P], f32, tag="at")
                    nc.vector.tensor_copy(out=at_sb[:Din, :], in_=at_ps[:Din, :])
                    z = ppool.tile([P, Dout], f32, tag="z")
                    nc.tensor.matmul(
                        out=z[:, :Dout], lhsT=at_sb[:Din, :], rhs=W_sb[:Din, :Dout],
                        start=True, stop=False,
                    )
                    nc.tensor.matmul(
                        out=z[:, :Dout], lhsT=bias_ap(jj),
                        rhs=b_sb[:1, :Dout], start=False, stop=True,
                    )
                    T = wpool.tile([P, Dout], out_dt, tag="t")
                    if relu:
                        nc.scalar.activation(
                            out=T[:, :Dout], in_=z[:, :Dout],
                            func=mybir.ActivationFunctionType.Relu,
                        )
                    elif softmax:
                        mneg = mpool.tile([P, 1], f32, tag="mneg")
                        nc.vector.tensor_reduce(
                            out=mneg[:, :], in_=z[:, :Dout],
                            axis=mybir.AxisListType.X, op=mybir.AluOpType.max,
                            negate=True,
                        )
                        nc.scalar.activation(
                            out=T[:, :Dout], in_=z[:, :Dout],
                            func=mybir.ActivationFunctionType.Exp,
                            bias=mneg[:, :1],
                        )
                        ssum = mpool.tile([P, 1], f32, tag="ssum")
                        nc.vector.tensor_reduce(
                            out=ssum[:, :], in_=T[:, :Dout],
                            axis=mybir.AxisListType.X, op=mybir.AluOpType.add,
                        )
                        rec = mpool.tile([P, 1], f32, tag="rec")
                        nc.vector.reciprocal(out=rec[:, :], in_=ssum[:, :])
                        nc.vector.tensor_scalar_mul(
                            out=T[:, :Dout], in0=T[:, :Dout], scalar1=rec[:, :1]
                        )
                    else:
                        nc.vector.tensor_copy(out=T[:, :Dout], in_=z[:, :Dout])
                    nc.sync.dma_start(
                        out=dst_dram[jj * P : (jj + 1) * P, :], in_=T[:, :Dout]
                    )

            dinvr_ap = lambda jj: dinvr[0:1, jj * P : (jj + 1) * P]
            ones_ap = lambda jj: ones_row[0:1, :]

            for _rep in range(REPS):
                prep_and_ag1()
                layer(t1, sl2, W1_sb, b1_sb, D0, D1, dinv2, dinvr_ap, True, False,
                      tdt, tdt)
                if ABLATE != "ag":
                    nc.gpsimd.collective_compute(
                        "AllGather", mybir.AluOpType.bypass, replica_groups=rg,
                        ins=[sl2[:, :]], outs=[t2[0:R, :]],
                    )
                layer(t2, sl3, W2_sb, b2_sb, D1, D2, dinv2, dinvr_ap, False, False,
                      tdt, td3)
                if ABLATE != "ag":
                    nc.gpsimd.collective_compute(
                        "AllGather", mybir.AluOpType.bypass, replica_groups=rg,
                        ins=[sl3[:, :]], outs=[t3[0:R, :]],
                    )
                layer(t3, out, W3_sb, b3_sb, D2, D3, dinv1, ones_ap, False, True,
                      td3, mybir.dt.float32)

    nc.compile()
    return nc


def make_runner(nc, n_cores=N_CORES):
    """Build the shard_map'd executable once; return (run_fn, time_fn).

    run_fn(in_maps) -> list of per-core output dicts (numpy).
    time_fn(in_maps, iters) -> list of per-iter wall seconds with inputs
    pre-placed on device (H2D excluded).
    """
    import jax
    import numpy as np2
    from jax.sharding import Mesh, PartitionSpec, NamedSharding
    from jax.experimental.shard_map import shard_map
    import concourse.mybir as mybir
    from concourse import bass2jax

    bass2jax.install_neuronx_cc_hook()

    in_names, out_names, out_avals, zero_outs = [], [], [], []
    for alloc in nc.m.functions[0].allocations:
        if not isinstance(alloc, mybir.MemoryLocationSet):
            continue
        name = alloc.memorylocations[0].name
        if alloc.kind == "ExternalInput":
            in_names.append(name)
        elif alloc.kind == "ExternalOutput":
            out_names.append(name)
            shape = tuple(alloc.tensor_shape)
            dtype = mybir.dt.np(alloc.dtype)
            out_avals.append(jax.core.ShapedArray(shape, dtype))
            zero_outs.append(np2.zeros(shape, dtype))
    partition_name = nc.partition_id_tensor.name if nc.partition_id_tensor else None
    if partition_name is not None and partition_name in in_names:
        in_names.remove(partition_name)
    n_params = len(in_names)
    n_outs = len(out_avals)
    all_in_names = in_names + out_names
    if partition_name is not None:
        all_in_names = all_in_names + [partition_name]

    def _body(*args):
        operands = list(args)
        if partition_name is not None:
            operands.append(bass2jax.partition_id_tensor())
        outs = bass2jax._bass_exec_p.bind(
            *operands,
            out_avals=tuple(out_avals),
            in_names=tuple(all_in_names),
            out_names=tuple(out_names),
            lowering_input_output_aliases=(),
            sim_require_finite=True,
            sim_require_nnan=True,
            nc=nc,
        )
        return tuple(outs)

    devices = jax.devices()[:n_cores]
    mesh = Mesh(np2.asarray(devices), ("core",))
    in_specs = (PartitionSpec("core"),) * (n_params + n_outs)
    out_specs = (PartitionSpec("core"),) * n_outs
    donate = tuple(range(n_params, n_params + n_outs))
    sharded = jax.jit(
        shard_map(_body, mesh=mesh, in_specs=in_specs, out_specs=out_specs,
                  check_rep=False),
        donate_argnums=donate, keep_unused=True,
    )
    sh = NamedSharding(mesh, PartitionSpec("core"))

    def _concat_inputs(in_maps):
        return [
            np2.concatenate([np2.asarray(in_maps[c][nm]) for c in range(n_cores)], axis=0)
            for nm in in_names
        ]

    def _zeros():
        return [np2.zeros((n_cores * z.shape[0], *z.shape[1:]), z.dtype)
                for z in zero_outs]

    def run_fn(in_maps):
        out_arrs = sharded(*_concat_inputs(in_maps), *_zeros())
        return [
            {nm: np2.asarray(out_arrs[i]).reshape(n_cores, *out_avals[i].shape)[c]
             for i, nm in enumerate(out_names)}
            for c in range(n_cores)
        ]

    def time_fn(in_maps, iters=5):
        import time as _t
        dev_in = [jax.device_put(a, sh) for a in _concat_inputs(in_maps)]
        for a in dev_in:
            a.block_until_ready()
        times = []
        for _ in range(iters):
            zs = [jax.device_put(z, sh) for z in _zeros()]
            for z in zs:
                z.block_until_ready()
            t0 = _t.time()
            outs = sharded(*dev_in, *zs)
            for o in outs:
                o.block_until_ready()
            times.append(_t.time() - t0)
        return times

    return run_fn, time_fn


# --------------------------------------------------------------------------
# Entry point
# --------------------------------------------------------------------------
def kernel(x, edge_index, W1, b1, W2, b2, W3, b3, _trace=False, _timed=0):
    from concourse.bass_utils import run_bass_kernel_spmd

    x = np.asarray(x, dtype=np.float32)
    W1 = np.asarray(W1, dtype=np.float32)
    W2 = np.asarray(W2, dtype=np.float32)
    W3 = np.asarray(W3, dtype=np.float32)
    b1 = np.asarray(b1, dtype=np.float32)
    b2 = np.asarray(b2, dtype=np.float32)
    b3 = np.asarray(b3, dtype=np.float32)
    n, D0 = x.shape
    D1 = W1.shape[1]
    D2 = W2.shape[1]
    D3 = W3.shape[1]

    pre, nc = build_all(edge_index, n, D0, D1, D2, D3)
    in_maps = shard_inputs(pre, x, W1, b1, W2, b2, W3, b3)

    if _timed:
        run_fn, time_fn = make_runner(nc)
        results = run_fn(in_maps)
        times = time_fn(in_maps, _timed)
    else:
        res = run_bass_kernel_spmd(
            nc, in_maps, core_ids=list(range(N_CORES)), trace=_trace
        )
        results = res.results
        times = None

    full = unshard(pre, results)
    if _timed:
        return full.astype(np.float32), times
    return full.astype(np.float32)


def build_all(edge_index, n, D0, D1, D2, D3):
    """Preprocess + build the Bass program per GATHER_MODE."""
    if GATHER_MODE == "bulk":
        pre = preprocess2(edge_index, n)
        nc = build_bass2(pre["J"], pre["K_A"], pre["K_B"], D0, D1, D2, D3,
                         bf16_tables=BF16_TABLES)
    else:
        pre = preprocess(edge_index, n)
        nc = build_bass(pre["J"], pre["K_u"], D0, D1, D2, D3,
                        bf16_tables=BF16_TABLES)
    return pre, nc


def shard_inputs(pre, x, W1, b1, W2, b2, W3, b3):
    J = pre["J"]
    D0 = x.shape[1]
    D1, D2, D3 = W1.shape[1], W2.shape[1], W3.shape[1]
    x_sh = np.zeros((N_CORES, J * P, D0), dtype=np.float32)
    x_sh[pre["core_of_node"], pre["local_of_node"]] = x
    idx_key = pre["idx_key"]
    idx_arr = pre["gi16"] if idx_key == "gi16" else pre["idx"]
    in_maps = []
    for c in range(N_CORES):
        in_maps.append(
            {
                "x_s": np.ascontiguousarray(x_sh[c]),
                "degt": np.ascontiguousarray(pre["deg_arr"][c]),
                "degr": np.ascontiguousarray(pre["deg_row"][c]),
                idx_key: np.ascontiguousarray(idx_arr[c]),
                "W1": np.asarray(W1, np.float32),
                "W2": np.asarray(W2, np.float32),
                "W3": np.asarray(W3, np.float32),
                "b1": np.asarray(b1, np.float32).reshape(1, D1),
                "b2": np.asarray(b2, np.float32).reshape(1, D2),
                "b3": np.asarray(b3, np.float32).reshape(1, D3),
            }
        )
    return in_maps


def unshard(pre, results):
    out_all = np.stack([results[c]["out"] for c in range(N_CORES)])
    return out_all[pre["core_of_node"], pre["local_of_node"]]

